# revision 23
# baseline (speedup 1.0000x reference)
"""Trainium2 Bass kernel for LayerNorm + multi-head attention + out-projection.

Reference computation (f32):
    h = LayerNorm(x) * ln_w + ln_b
    q, k, v = split(h @ w_qkv)          # 16 heads, head_dim 64
    out = softmax(q k^T / 8) v          # per head, full 2048-seq attention
    return concat_heads(out) @ w_out
Sharding over 8 NeuronCores: core c -> (batch b = c // 2, head-group g = c % 2).
Each core handles one batch and 8 of the 16 heads (tensor parallel on heads:
w_qkv column-split, w_out row-split).  Each core emits a partial [2048, 1024]
output; the host sums the two partials of each batch.

Single merged pipeline (no phase split): the 32 attention units (head-pair j
outer, q-block qb middle, head hh inner) run back to back; ALL projection work
(kT, qT, per-pair V, out-proj) is deferred into per-unit slots between the S
matmul pairs, emitted just before its first consumer, so the exp-bound ACT
engine and the matmul-bound PE both stay ~fully busy from ~15us on.
j-major unit order means pair j's kT/qT/V are produced during pair j-1's
units; the first exps fire as soon as block 0's LayerNorm + kT[0]b0 land.

Device-side dataflow per core (all matmuls out = lhsT.T @ rhs):
  - x arrives bf16 (host-converted; LN stats in f32 internally).  LayerNorm
    token-major (bn_stats/bn_aggr on DVE, affine apply on GPSIMD, bf16 h),
    then DMA-XBAR transpose h -> hT [d-part, tokens].
  - kT/qT per 512-token block (bf16); V per (head-pair, token-tile) piece
    with an extra ones column per head (accumulates the softmax denominator
    during AV).
  - Attention per (j, qb, hh): S^T = kT.T @ qT into [128 ktok, 1024] PSUM
    tiles -> 1024-wide exp on ScalarE (1/8 scale fused; S ~ N(0,1) so no max
    subtraction) -> bf16 P^T tiles in SBUF.  AV is "flipped": P^T tiles are
    the stationary operand and V (65 cols, with ones) is the moving operand,
    accumulating [128 qtok, 65] PSUM per q-tile over the 16 k-tiles.  Row 64
    is the denominator; normalization is a per-partition reciprocal +
    tensor_scalar into token-major o tiles.
  - o tiles [128 qtok, 128] are DMA-XBAR transposed into oT [inner, tok];
    out = oT.T @ Wout streamed to DRAM (bf16 partials; host sums in f32).

The LayerNorm affine is folded into the projections host-side (exact):
h @ W = ((x - mu) * rstd) @ (diag(ln_w) W) + ln_b @ W, so the device only
computes (x - mu) * rstd and adds the ln_b @ W bias during the PSUM->SBUF
copy of each projection.

Engine budget per core (cost model): PE ~270 us and ACT ~271 us busy, both
~paced; e2e target ~295 us (baseline two-phase version: 365 us, HW-validated
rel err 5.3e-3).
"""

from collections import defaultdict
from contextlib import ExitStack

import numpy as np

import concourse.bass as bass
import concourse.tile as tile
from concourse import bacc, masks, mybir

import ml_dtypes

P = 128
EPS = 1e-5


def _bcast_partition(ap, n, skip_partition=True):
    """AP that reads a [1, F] access pattern broadcast to [n, F] partitions."""
    dims = list(ap.ap[1:]) if skip_partition else list(ap.ap)
    if skip_partition:
        part = list(ap.ap[0])
        return bass.AP(tensor=ap.tensor, offset=ap.offset,
                       ap=[[part[0], 1], [0, n]] + dims)
    return bass.AP(tensor=ap.tensor, offset=ap.offset, ap=[[0, n]] + dims)


def emit_body(ctx, tc, io, ntok, d, nh, hd, repeat=1):
    nc = tc.nc
    f32 = mybir.dt.float32
    bf16 = mybir.dt.bfloat16
    Act = mybir.ActivationFunctionType
    Alu = mybir.AluOpType

    cc = nh * hd            # head cols per core (512)
    n_dt = d // P           # d-model tiles (8)
    n_tt = ntok // P        # token tiles (16)
    FQ = min(512, ntok)     # q block size
    n_qb = ntok // FQ       # q blocks (4)
    n_ct = cc // P          # head-pair tiles (4)
    tpb = FQ // P           # token tiles per block (4)
    n_bl = n_tt // tpb      # 512-token blocks (4)
    n_k2 = n_tt // 2        # kt-pair count (8)
    FN = min(512, d)        # out-proj free block
    n_nb = d // FN          # out-proj col blocks (2)
    bn_ch = min(512, d)     # bn_stats chunk size
    n_ch = d // bn_ch       # bn_stats chunks (2)
    vw = hd + 1             # V cols per head incl. ones column (65)
    scale = float(hd) ** -0.5

    x_d, wq_d, wk_d, wv_d, wo_d, bq_d, bk_d, bv_d, out_d = io

    # ---------------- constants & weights ----------------
    const = ctx.enter_context(tc.tile_pool(name="const", bufs=1))
    eps_sb = const.tile([P, 1], f32)
    nc.vector.memset(eps_sb[:], EPS)
    bq_sb = const.tile([P, n_ct], f32)
    bk_sb = const.tile([P, n_ct], f32)
    bv_bc = const.tile([P, cc], f32)
    identity = const.tile([P, P], bf16)
    masks.make_identity(nc, identity[:])

    wpool = ctx.enter_context(tc.tile_pool(name="weights", bufs=1))
    wq_sb = wpool.tile([P, n_dt, cc], bf16)
    wk_sb = wpool.tile([P, n_dt, cc], bf16)
    wv_sb = wpool.tile([P, n_dt, cc], bf16)
    wo_sb = wpool.tile([P, n_ct, d], bf16)

    # ---------------- persistent activations ----------------
    big = ctx.enter_context(tc.tile_pool(name="big", bufs=1))
    hT = big.tile([P, n_dt, ntok], bf16, tag="hT", name="hT")
    qT = [big.tile([P, ntok], bf16, tag=f"qT{j}", name=f"qT{j}") for j in range(n_ct)]
    kT = [big.tile([P, ntok], bf16, tag=f"kT{j}", name=f"kT{j}") for j in range(n_ct)]
    # V as per-(pair, token-tile) tiles [P, 2*vw]: layout [h0 data | ones |
    # h1 data | ones]; separate tiles keep pair-j writes during attention
    # from creating any ordering hazard with pair-i reads.
    V = [[big.tile([P, 2 * vw], bf16, tag=f"V{j2}_{t}", name=f"V{j2}_{t}")
          for t in range(n_tt)] for j2 in range(n_ct)]
    oT = big.tile([P, n_ct, ntok], bf16, tag="oT", name="oT")

    # exp (P^T) tiles: ring sized for current unit (8) + pending unit (8)
    expool = ctx.enter_context(tc.tile_pool(name="expool", bufs=16))

    # ---------------- PSUM pools (8 banks total, persistent) -------------
    # pss: 3 x [P,1024] f32 (2 banks each) so the PE can run up to 3 S
    # chunks ahead of the exp drain; paux: shared 2-ring for every other
    # PSUM user (projections, AV, out-proj, hT transposes).
    pss_p = ctx.enter_context(tc.tile_pool(name="pss", bufs=3, space="PSUM"))
    paux_p = ctx.enter_context(tc.tile_pool(name="paux", bufs=2, space="PSUM"))
    psq_p = paux_p
    p1b_p = paux_p

    # small SBUF pools
    osml_p = ctx.enter_context(tc.tile_pool(name="osml", bufs=16))
    rs_p = ctx.enter_context(tc.tile_pool(name="rsp", bufs=12))
    out_p = ctx.enter_context(tc.tile_pool(name="outp", bufs=4))

    def s_exp(j, qb, off, kt2):
        """S^T for ktile pair kt2 (one head) -> exp -> bf16 SBUF tile."""
        pss = pss_p.tile([P, 2 * FQ], f32, tag="pss")
        for u in range(2):
            kt = 2 * kt2 + u
            nc.tensor.matmul(pss[:, u * FQ:(u + 1) * FQ],
                             lhsT=kT[j][off:off + hd, kt * P:(kt + 1) * P],
                             rhs=qT[j][off:off + hd, qb * FQ:(qb + 1) * FQ],
                             start=True, stop=True)
        ex = expool.tile([P, 2 * FQ], bf16, tag="ex")
        nc.scalar.activation(ex[:], pss[:], Act.Exp, scale=scale)
        return ex

    def kq_one(dst, w_sb, b_sb, j, b):
        ps = psq_p.tile([P, FQ], f32, tag="paux")
        for k in range(n_dt):
            nc.tensor.matmul(ps[:], lhsT=w_sb[:, k, j * P:(j + 1) * P],
                             rhs=hT[:, k, b * FQ:(b + 1) * FQ],
                             start=(k == 0), stop=(k == n_dt - 1))
        nc.vector.tensor_scalar_add(out=dst[j][:, b * FQ:(b + 1) * FQ],
                                    in0=ps[:], scalar1=b_sb[:, j:j + 1])

    def v_piece(j2, t):
        """V projection for head-pair j2, token tile t (128 free cols)."""
        vv = V[j2][t][:].rearrange("p (h c) -> p h c", c=vw)   # [P, 2, 65]
        nc.vector.memset(vv[:, :, hd:hd + 1], 1.0)
        psv = psq_p.tile([P, 2 * hd], f32, tag="paux")
        for k in range(n_dt):
            nc.tensor.matmul(psv[:], lhsT=hT[:, k, t * P:(t + 1) * P],
                             rhs=wv_sb[:, k, 2 * j2 * hd:(2 * j2 + 2) * hd],
                             start=(k == 0), stop=(k == n_dt - 1))
        nc.vector.tensor_add(vv[:, :, 0:hd],
                             psv[:].rearrange("p (h c) -> p h c", c=hd),
                             bv_bc[:, 2 * j2 * hd:(2 * j2 + 2) * hd]
                             .rearrange("p (h c) -> p h c", c=hd))

    osm = {}        # (j, qt) -> o tile of the current qb

    def av_norm(qb, j, hh, exs, qt):
        off = hh * hd
        if hh == 0:
            osm[(j, qt)] = osml_p.tile([P, P], bf16, tag="osml",
                                       name=f"osm{qb}_{j}_{qt}")
        pav = p1b_p.tile([P, FQ], f32, tag="paux")
        for kt in range(n_tt):
            nc.tensor.matmul(
                pav[:, 0:vw],
                lhsT=exs[kt // 2][:, (kt % 2) * FQ + qt * P:
                                  (kt % 2) * FQ + (qt + 1) * P],
                rhs=V[j][kt][:, hh * vw:(hh + 1) * vw],
                start=(kt == 0), stop=(kt == n_tt - 1))
        rec = rs_p.tile([P, 1], f32, tag="rec")
        nc.vector.reciprocal(rec[:], pav[:, hd:hd + 1])
        nc.vector.tensor_scalar_mul(
            out=osm[(j, qt)][:, off:off + hd],
            in0=pav[:, 0:hd], scalar1=rec[:, 0:1])
        if hh == 1:
            nc.scalar.dma_start_transpose(
                oT[:, j, (qb * tpb + qt) * P:(qb * tpb + qt + 1) * P],
                osm[(j, qt)][:])

    def outproj_part(qb, part):
        tt = qb * tpb + part
        for nb in range(n_nb):
            ps = p1b_p.tile([P, FN], f32, tag="paux")
            for j2 in range(n_ct):
                nc.tensor.matmul(ps[:], lhsT=oT[:, j2, tt * P:(tt + 1) * P],
                                 rhs=wo_sb[:, j2, nb * FN:(nb + 1) * FN],
                                 start=(j2 == 0), stop=(j2 == n_ct - 1))
            ot = out_p.tile([P, FN], bf16, tag="ot")
            nc.vector.tensor_copy(ot[:], ps[:])
            nc.sync.dma_start(
                out=out_d[tt * P:(tt + 1) * P, nb * FN:(nb + 1) * FN],
                in_=ot[:])

    # ---------------- unit schedule (j-major) ----------------
    # units[u] = (j, qb, hh); u0 is emitted inline with the LayerNorm
    # prologue; units 1..31 go through unit_step with the extras table.
    units = [(j, qb, hh) for j in range(n_ct)
             for qb in range(n_qb) for hh in range(2)]

    # extras[u] -> closures emitted as one contiguous always-ready PE burst
    # at the start of unit u (before the ACT-paced S chunks), so the PE runs
    # long uninterrupted stretches and holds its ramped p-state.  late[u]
    # runs after unit u's AV groups (out-proj parts needing this unit's oT).
    extras = defaultdict(list)
    for p in range(1, n_ct):
        base = 8 * (p - 1)
        for i in range(8):
            extras[base + 1 + i].append(lambda p=p, t=2 * i: v_piece(p, t))
            extras[base + 1 + i].append(lambda p=p, t=2 * i + 1: v_piece(p, t))
        if p == 1:
            continue          # kT[1] + qT[1][qb0] are emitted in the prologue
        for b in range(n_bl):
            extras[base + 4 + b].append(
                lambda p=p, b=b: kq_one(kT, wk_sb, bk_sb, p, b))
        extras[base + 7].append(
            lambda p=p: kq_one(qT, wq_sb, bq_sb, p, 0))
    for j in range(n_ct):
        for qb2 in range(1, n_qb):
            extras[8 * j + 2 * qb2 - 1].append(
                lambda j=j, qb2=qb2: kq_one(qT, wq_sb, bq_sb, j, qb2))
    late = defaultdict(list)
    for Y in range(n_qb - 1):
        uu = 8 * (n_ct - 1) + 2 * (Y + 1)        # 26, 28, 30
        for part in range(2):
            late[uu].append(lambda Y=Y, part=part: outproj_part(Y, part))
            late[uu + 1].append(
                lambda Y=Y, part=part: outproj_part(Y, part + 2))

    pending = None      # ((qb, j, hh), exs) whose AV is not yet emitted

    def unit_step(u):
        nonlocal pending
        j, qb, hh = units[u]
        for fn in extras.get(u, []):
            fn()
        exs = []
        for kt2 in range(n_k2):
            exs.append(s_exp(j, qb, hh * hd, kt2))
            if pending is not None and kt2 >= n_k2 // 2:
                # back-half slots: the pending unit's last exp finished an
                # entire half-unit ago, so these AV chains never wait on ACT
                (pqb, pj, phh), pexs = pending
                av_norm(pqb, pj, phh, pexs, kt2 - n_k2 // 2)
        for fn in late.get(u, []):
            fn()
        pending = ((qb, j, hh), exs)

    # ---------------- prologue: LN pipeline + unit 0 ----------------
    # DMA track is an exclusive resource: order transfers by first use
    # (x b0 -> wk -> bq/bk -> x b1 -> wq -> x b2/b3 -> wv -> bv -> wo).
    # All LN sqrts are emitted before the first exp so the ACT table is
    # switched Sqrt->Exp exactly once (1283 ns per table load).
    # hT transposes run on the PE (128x128 via identity, 53 ns each) into a
    # single-bank bf16 PSUM stage, drained 1024-wide by DVE -- keeping the
    # DMA track free for the x/weight streams.
    with tc.tile_pool(name="xin", bufs=11) as xin_p, \
         tc.tile_pool(name="hnat", bufs=5) as h_p, \
         tc.tile_pool(name="stats", bufs=10) as st_p:

        hts = {}
        xts = {}

        def emit_x(b):
            for t in range(b * tpb, (b + 1) * tpb):
                xt = xts[t] = xin_p.tile([P, d], bf16, tag="xt",
                                         name=f"xt{t}")
                nc.sync.dma_start(out=xt[:], in_=x_d[t * P:(t + 1) * P, :])

        def ln_compute(b):
            mvs, rstds = {}, {}
            for t in range(b * tpb, (b + 1) * tpb):
                xt = xts[t]
                st = st_p.tile([P, n_ch, 6], f32, tag="st")
                for c in range(n_ch):
                    nc.vector.bn_stats(st[:, c, :], xt[:, c * bn_ch:(c + 1) * bn_ch])
                mv = mvs[t] = st_p.tile([P, 2], f32, tag="mv", name=f"mv{t}")
                nc.vector.bn_aggr(mv[:], st[:])
            for t in range(b * tpb, (b + 1) * tpb):
                rstd = rstds[t] = st_p.tile([P, 1], f32, tag="rstd",
                                            name=f"rstd{t}")
                nc.scalar.activation(rstd[:], mvs[t][:, 1:2], Act.Sqrt,
                                     bias=eps_sb[:], scale=1.0)
            for t in range(b * tpb, (b + 1) * tpb):
                nc.vector.reciprocal(rstds[t][:], rstds[t][:])
            for t in range(b * tpb, (b + 1) * tpb):
                xt, mv, rstd = xts.pop(t), mvs[t], rstds[t]
                ht = hts[t] = h_p.tile([P, d], bf16, tag="ht", name=f"ht{t}")
                half = d // 2
                for c2 in range(2):
                    nc.gpsimd.tensor_scalar(out=ht[:, c2 * half:(c2 + 1) * half],
                                            in0=xt[:, c2 * half:(c2 + 1) * half],
                                            scalar1=mv[:, 0:1], scalar2=rstd[:],
                                            op0=Alu.subtract, op1=Alu.mult)

        def transpose_block(b):
            # block 0 via PE transpose + ACT copy (earliest need, ACT idle);
            # blocks 1-2 via DMA XBAR (track has slack there); block 3 via
            # PE transpose + DVE copy (DVE frees up right when b3 lands).
            for t in range(b * tpb, (b + 1) * tpb):
                ht = hts.pop(t)
                if b == 1:
                    nc.sync.dma_start_transpose(hT[:, :, t * P:(t + 1) * P],
                                                ht[:])
                    continue
                pst = p1b_p.tile([P, n_dt, P], bf16, tag="paux")
                for k in range(n_dt):
                    nc.tensor.transpose(pst[:, k, :], ht[:, k * P:(k + 1) * P],
                                        identity[:])
                if b == 0:
                    nc.scalar.copy(hT[:, :, t * P:(t + 1) * P], pst[:])
                else:
                    nc.vector.tensor_copy(hT[:, :, t * P:(t + 1) * P], pst[:])

        emit_x(0)
        nc.sync.dma_start(out=wk_sb[:], in_=wk_d.rearrange("(k p) c -> p k c", p=P))
        nc.sync.dma_start(out=bq_sb[:], in_=bq_d.rearrange("(j p) -> p j", p=P))
        nc.sync.dma_start(out=bk_sb[:], in_=bk_d.rearrange("(j p) -> p j", p=P))
        emit_x(1)
        nc.sync.dma_start(out=wq_sb[:], in_=wq_d.rearrange("(k p) c -> p k c", p=P))
        emit_x(2)
        emit_x(3)
        nc.sync.dma_start(out=wv_sb[:], in_=wv_d.rearrange("(k p) c -> p k c", p=P))
        nc.gpsimd.dma_start(out=bv_bc[:],
                            in_=_bcast_partition(bv_d, P, skip_partition=False))
        nc.sync.dma_start(out=wo_sb[:], in_=wo_d.rearrange("(j p) c -> p j c", p=P))
        exs0 = []
        ln_compute(0)
        transpose_block(0)                # PE transp + ACT copies (~t=9)
        ln_compute(1)
        kq_one(kT, wk_sb, bk_sb, 0, 0)    # kT[0] b0
        kq_one(qT, wq_sb, bq_sb, 0, 0)    # qT[0] qb0
        exs0.append(s_exp(0, 0, 0, 0))    # unit 0 = (j0, qb0, h0)
        exs0.append(s_exp(0, 0, 0, 1))
        for t in range(0, 4):             # b0-dependent filler while the
            v_piece(0, t)                 # b1 DMA transposes land
        kq_one(kT, wk_sb, bk_sb, 1, 0)
        kq_one(qT, wq_sb, bq_sb, 1, 0)
        transpose_block(1)                # DMA XBAR
        ln_compute(2)
        kq_one(kT, wk_sb, bk_sb, 0, 1)
        exs0.append(s_exp(0, 0, 0, 2))
        exs0.append(s_exp(0, 0, 0, 3))
        for t in range(4, 8):
            v_piece(0, t)
        kq_one(kT, wk_sb, bk_sb, 1, 1)
        ln_compute(3)
        transpose_block(2)                # PE transp + DVE copies (~t=25)
        kq_one(kT, wk_sb, bk_sb, 0, 2)
        exs0.append(s_exp(0, 0, 0, 4))
        exs0.append(s_exp(0, 0, 0, 5))
        for t in range(8, 12):
            v_piece(0, t)
        kq_one(kT, wk_sb, bk_sb, 1, 2)
        transpose_block(3)                # PE transp + DVE copies
        kq_one(kT, wk_sb, bk_sb, 0, 3)
        exs0.append(s_exp(0, 0, 0, 6))
        exs0.append(s_exp(0, 0, 0, 7))
        for t in range(12, n_tt):
            v_piece(0, t)
        kq_one(kT, wk_sb, bk_sb, 1, 3)
        pending = ((0, 0, 0), exs0)

    # ---------------- main loop + tail ----------------
    for u in range(1, len(units)):
        unit_step(u)
    (pqb, pj, phh), pexs = pending        # = (qb3, j3, h1)
    for qt in range(tpb):
        av_norm(pqb, pj, phh, pexs, qt)
    for qt in range(tpb):
        outproj_part(n_qb - 1, qt)


def build_nc(ntok=2048, d=1024, nh=8, hd=64, n_cores=8, repeat=1):
    nc = bacc.Bacc("TRN2", target_bir_lowering=False, debug=False,
                   num_devices=n_cores)
    f32 = mybir.dt.float32
    bf16 = mybir.dt.bfloat16
    cc = nh * hd
    x_d = nc.dram_tensor("x", [ntok, d], bf16, kind="ExternalInput").ap()
    wq_d = nc.dram_tensor("wq", [d, cc], bf16, kind="ExternalInput").ap()
    wk_d = nc.dram_tensor("wk", [d, cc], bf16, kind="ExternalInput").ap()
    wv_d = nc.dram_tensor("wv", [d, cc], bf16, kind="ExternalInput").ap()
    wo_d = nc.dram_tensor("wo", [cc, d], bf16, kind="ExternalInput").ap()
    bq_d = nc.dram_tensor("bq", [cc], f32, kind="ExternalInput").ap()
    bk_d = nc.dram_tensor("bk", [cc], f32, kind="ExternalInput").ap()
    bv_d = nc.dram_tensor("bv", [cc], f32, kind="ExternalInput").ap()
    out_d = nc.dram_tensor("out", [ntok, d], bf16, kind="ExternalOutput").ap()
    io = (x_d, wq_d, wk_d, wv_d, wo_d, bq_d, bk_d, bv_d, out_d)
    with tile.TileContext(nc) as tc:
        with ExitStack() as ctx:
            emit_body(ctx, tc, io, ntok, d, nh, hd, repeat=repeat)
    nc.compile()
    return nc


_CACHE = {}


def _make_runner(nc, n_cores):
    """Build a reusable sharded PJRT callable for the compiled Bass module."""
    import jax
    from jax.sharding import Mesh, PartitionSpec
    from jax.experimental.shard_map import shard_map
    from concourse.bass2jax import (_bass_exec_p, install_neuronx_cc_hook,
                                    partition_id_tensor)

    install_neuronx_cc_hook()
    partition_name = (nc.partition_id_tensor.name
                      if nc.partition_id_tensor else None)

    in_names, out_names, out_avals = [], [], []
    for alloc in nc.m.functions[0].allocations:
        if not isinstance(alloc, mybir.MemoryLocationSet):
            continue
        name = alloc.memorylocations[0].name
        if alloc.kind == "ExternalInput":
            if name != partition_name:
                in_names.append(name)
        elif alloc.kind == "ExternalOutput":
            out_names.append(name)
            out_avals.append(jax.core.ShapedArray(
                tuple(alloc.tensor_shape), mybir.dt.np(alloc.dtype)))
    all_names = in_names + out_names
    if partition_name is not None:
        all_names = all_names + [partition_name]

    def _body(*args):
        operands = list(args)
        if partition_name is not None:
            operands.append(partition_id_tensor())
        outs = _bass_exec_p.bind(
            *operands,
            out_avals=tuple(out_avals),
            in_names=tuple(all_names),
            out_names=tuple(out_names),
            lowering_input_output_aliases=(),
            sim_require_finite=True,
            sim_require_nnan=True,
            nc=nc,
        )
        return tuple(outs)

    devices = jax.devices()[:n_cores]
    assert len(devices) == n_cores
    mesh = Mesh(np.asarray(devices), ("core",))
    nio = len(in_names) + len(out_names)
    sharded = jax.jit(
        shard_map(_body, mesh=mesh,
                  in_specs=(PartitionSpec("core"),) * nio,
                  out_specs=(PartitionSpec("core"),) * len(out_names),
                  check_rep=False),
        keep_unused=True)
    return sharded, in_names, out_names, out_avals


def _concat_inputs(in_maps, in_names, out_avals, n_cores):
    concat_in = [np.concatenate([np.asarray(in_maps[c][name])
                                 for c in range(n_cores)], axis=0)
                 for name in in_names]
    concat_zeros = [np.zeros((n_cores * a.shape[0], *a.shape[1:]), a.dtype)
                    for a in out_avals]
    return concat_in + concat_zeros


def _run_spmd(in_maps, n_cores):
    sharded, in_names, out_names, out_avals = _CACHE["runner"]
    args = _concat_inputs(in_maps, in_names, out_avals, n_cores)
    _CACHE["last_args"] = args
    out_arrs = sharded(*args)
    return [
        {name: np.asarray(out_arrs[i]).reshape(n_cores, *out_avals[i].shape)[c]
         for i, name in enumerate(out_names)}
        for c in range(n_cores)
    ]


def kernel(x, ln_w, ln_b, w_qkv, w_out):
    x = np.asarray(x, dtype=np.float32)
    ln_w = np.asarray(ln_w, dtype=np.float32)
    ln_b = np.asarray(ln_b, dtype=np.float32)
    w_qkv = np.asarray(w_qkv, dtype=np.float32)
    w_out = np.asarray(w_out, dtype=np.float32)

    B, ntok, d = x.shape               # 4, 2048, 1024
    inner = w_out.shape[0]             # 1024
    hd = 64
    H = inner // hd                    # 16
    n_cores = 8
    gpb = n_cores // B                 # head-groups per batch (2)
    nh = H // gpb                      # heads per core (8)
    cc = nh * hd                       # 512

    if "nc" not in _CACHE:
        _CACHE["nc"] = build_nc(ntok=ntok, d=d, nh=nh, hd=hd, n_cores=n_cores)
    nc = _CACHE["nc"]

    bf = ml_dtypes.bfloat16
    # fold the LayerNorm affine into the projections (exact):
    #   h = (x - mu) * rstd * ln_w + ln_b
    #   h @ W = ((x - mu) * rstd) @ (diag(ln_w) W) + (ln_b @ W)
    wq_f = ln_w[:, None] * w_qkv[:, 0 * inner:1 * inner]
    wk_f = ln_w[:, None] * w_qkv[:, 1 * inner:2 * inner]
    wv_f = ln_w[:, None] * w_qkv[:, 2 * inner:3 * inner]
    bq_f = ln_b @ w_qkv[:, 0 * inner:1 * inner]
    bk_f = ln_b @ w_qkv[:, 1 * inner:2 * inner]
    bv_f = ln_b @ w_qkv[:, 2 * inner:3 * inner]

    in_maps = []
    for c in range(n_cores):
        b, g = divmod(c, gpb)
        cols = slice(g * cc, (g + 1) * cc)
        in_maps.append({
            "x": np.ascontiguousarray(x[b]).astype(bf),
            "wq": np.ascontiguousarray(wq_f[:, cols]).astype(bf),
            "wk": np.ascontiguousarray(wk_f[:, cols]).astype(bf),
            "wv": np.ascontiguousarray(wv_f[:, cols]).astype(bf),
            "wo": np.ascontiguousarray(w_out[cols, :]).astype(bf),
            "bq": np.ascontiguousarray(bq_f[cols]).astype(np.float32),
            "bk": np.ascontiguousarray(bk_f[cols]).astype(np.float32),
            "bv": np.ascontiguousarray(bv_f[cols]).astype(np.float32),
        })

    if "runner" not in _CACHE:
        _CACHE["runner"] = _make_runner(nc, n_cores)
    results = _run_spmd(in_maps, n_cores)
    parts = [results[c]["out"].astype(np.float32) for c in range(n_cores)]
    out = np.stack([sum(parts[b * gpb + g] for g in range(gpb))
                    for b in range(B)])
    return out.astype(np.float32)
